# revision 116
# baseline (speedup 1.0000x reference)
"""Deformable-conv (DCNv1) Trainium2 Bass kernel, 8-way sharded.

Shapes (hardcoded from the problem spec):
  x  [2, 64, 128, 128] f32   input image
  Wp [18, 64, 3, 3]    f32   offset-conv weights (2*9 offset channels)
  bp [18]              f32   offset-conv bias
  Wc [64, 64, 3, 3]    f32   final conv weights (stride-3 over unfolded samples)
  out [2, 64, 128, 128] f32

Sharding: 8 cores; core k handles batch k//4, output rows (k%4)*32 .. +32.

Per-core pipeline (bf16 data path, f32 offset/index math). The critical
resource is the gather-descriptor spine (8 x ~7.7us of indirect-DMA); all
compute is spread across DVE / Act / Pool / PE to hide under it:
  1. offset conv directly pixel-major: per row, 9 accumulating matmuls
     (lhsT = x window [65c,128w] incl. a ones-row, rhs = Wp tap [65c,18] with
     the bias folded into tap 0) -> PSUM [128w, (j,18)]; positions read PSUM.
  2. positions px/py, clip, floor (round + is_gt fixup: HW f32->i32 rounds),
     gather index idx = floor_row*130 + floor_col on DVE; index wrap to the
     [i%16, i/16]-replicated layout via 2 PE transpose stages + selector
     matmul, PSUM->SBUF moves on Act.
  3. ONE indirect-DMA gather per 4-row group: 128*36 rows x 256 bf16 from the
     host-prepacked difference table xt[16900, 256] where each padded pixel's
     row is [a | b=right-a | c=down-a | d=a+diag-right-down] x 64 channels.
  4. bilinear weights wt[p,s,k,0:16] = (fc, fr, fr*fc) built by log-doubling
     (4x bf16 copies); one batched 2x TT multiply per j-row c-quarter forms
     pp = lanes * weights (lane-major); the a-lane is a 4x copy.
  5. the 4-corner reduction rides the PE for free: per 128-col chunk, 4
     accumulating regular matmuls (lhsT=lane chunk, rhs=identity; the HW
     transpose datapath does NOT accumulate, regular matmuls do) produce
     xoT chunks in PSUM; Act copies them to SBUF; the final conv accumulates
     5 chunks into the same PSUM tile; per-row DMA out.
  Last group: Pool (idle after its final gather) takes the j=3 combine and
  DVE the PSUM copies to shorten the pipeline drain.

Clipping exactness: the reference clips corner indices into [0,129] of the
zero-padded image, so every clipped corner reads a zero; unclipped fractions
with clipped indices reproduce it exactly. Table right/down neighbors at
index 129 are zero-extended (pad to 131), matching the reference's clip.
"""

import numpy as np
import ml_dtypes

import concourse.bacc as bacc
import concourse.bass as bass
import concourse.tile as tile
from concourse import mybir
from concourse.bass_utils import run_bass_kernel_spmd
from concourse.masks import make_identity

B, C, H, W, OUTC = 2, 64, 128, 128, 64
KS, N = 3, 9
PADH = H + 2  # 130
TROWS = PADH * PADH  # 16900 table rows
ROWS_PER_CORE = 32
NGROUP = 8          # groups of 4 output rows per core
JROWS = 4
NSAMP = JROWS * N   # 36 samples per w-pixel per group
XS_ROWS = ROWS_PER_CORE + 2

F32 = mybir.dt.float32
BF16 = mybir.dt.bfloat16
I32 = mybir.dt.int32
OP = mybir.AluOpType

import os
DEBUG_DUMP = bool(int(os.environ.get("DCN_DEBUG", "0")))



_CACHE = {}


def _build_nc():
    nc = bacc.Bacc("TRN2", target_bir_lowering=False, debug=False,
                   dynamic_dma_scratch_size=36864, num_swdge_queues=2)

    xt = nc.dram_tensor("xt", [TROWS, 2 * C], I32, kind="ExternalInput")
    # xs/wp carry an extra ones/bias row so the offset conv includes the bias
    xs = nc.dram_tensor("xs", [C + 1, XS_ROWS * PADH], BF16, kind="ExternalInput")
    wp = nc.dram_tensor("wp", [C + 1, N * 2 * N], BF16, kind="ExternalInput")
    cpx = nc.dram_tensor("cpx", [128, NSAMP], F32, kind="ExternalInput")
    cpy = nc.dram_tensor("cpy", [128, NSAMP], F32, kind="ExternalInput")
    wc2 = nc.dram_tensor("wc2", [2 * C, 4 * OUTC], BF16, kind="ExternalInput")
    wc8 = nc.dram_tensor("wc8", [C, OUTC], BF16, kind="ExternalInput")
    sel = nc.dram_tensor("sel", [16, 128], F32, kind="ExternalInput")
    out = nc.dram_tensor("out", [OUTC, ROWS_PER_CORE * W], F32, kind="ExternalOutput")

    dbg = {}
    if DEBUG_DUMP:
        dbg["idx16"] = nc.dram_tensor("d_idx16", [128, 8 * NSAMP], mybir.dt.int16, kind="ExternalOutput")
        dbg["gath"] = nc.dram_tensor("d_gath", [128, NSAMP * 4 * C], BF16, kind="ExternalOutput")
        dbg["wt"] = nc.dram_tensor("d_wt", [128, NSAMP * 3 * 16], BF16, kind="ExternalOutput")
        dbg["pp"] = nc.dram_tensor("d_pp", [128, 4 * NSAMP * C], BF16, kind="ExternalOutput")
        dbg["xot"] = nc.dram_tensor("d_xot", [128, 4 * 128], BF16, kind="ExternalOutput")

    with tile.TileContext(nc) as tc:
        _emit(tc, nc, xt, xs, wp, cpx, cpy, wc2, wc8, sel, out, dbg)
    nc.compile()
    return nc


def _emit(tc, nc, xt, xs, wp, cpx, cpy, wc2, wc8, sel, out, dbg=None):
    from contextlib import ExitStack

    with ExitStack() as ctx:
        const = ctx.enter_context(tc.tile_pool(name="const", bufs=1))
        sb = ctx.enter_context(tc.tile_pool(name="sb", bufs=2))
        smalls = ctx.enter_context(tc.tile_pool(name="smalls", bufs=3))
        tmp = ctx.enter_context(tc.tile_pool(name="tmp", bufs=4))
        gpool = ctx.enter_context(tc.tile_pool(name="gpool", bufs=4))
        ps_conv = ctx.enter_context(tc.tile_pool(name="ps_conv", bufs=2, space="PSUM"))
        ps_xot = ctx.enter_context(tc.tile_pool(name="ps_xot", bufs=2, space="PSUM"))
        ps_idx = ctx.enter_context(tc.tile_pool(name="ps_idx", bufs=1, space="PSUM"))

        # ---- constants resident in SBUF ----
        ident = const.tile([128, 128], BF16)
        make_identity(nc, ident[:])
        identf = const.tile([128, 128], F32)
        make_identity(nc, identf[:])
        # const loads split across two DGE queues (SP + Act) to shorten ramp-in;
        # xs loads in two pieces so group 0's conv starts after the first rows
        xs_sb = const.tile([C + 1, XS_ROWS * PADH], BF16)
        nc.sync.dma_start(xs_sb[:, 0:6 * PADH], xs[:, 0:6 * PADH])
        cpx_sb = const.tile([128, NSAMP], F32)
        nc.sync.dma_start(cpx_sb[:], cpx[:])
        cpy_sb = const.tile([128, NSAMP], F32)
        nc.sync.dma_start(cpy_sb[:], cpy[:])
        wp_sb = const.tile([C + 1, N * 2 * N], BF16)
        nc.scalar.dma_start(wp_sb[:], wp[:])
        sel_sb = const.tile([16, 128], F32)
        nc.scalar.dma_start(sel_sb[:], sel[:])
        nc.sync.dma_start(xs_sb[:, 6 * PADH:], xs[:, 6 * PADH:])
        wc2_sb = const.tile([2 * C, 4 * OUTC], BF16)
        nc.sync.dma_start(wc2_sb[:], wc2[:])
        wc8_sb = const.tile([C, OUTC], BF16)
        nc.sync.dma_start(wc8_sb[:], wc8[:])

        for g in range(NGROUP):
            # ---- 1. offset conv for 4 rows -> PSUM [128 w, (j, 18 chan)] ----
            # lhsT = x window [64c, 128w], rhs = Wp tap [64c, 18] so the
            # output lands pixel-major: no transpose needed.
            conv_ps = ps_conv.tile([128, JROWS * 2 * N], F32)
            for j in range(JROWS):
                row = JROWS * g + j  # local row; padded source row = row + ti
                for t in range(N):
                    ti, tj = t // 3, t % 3
                    base = (row + ti) * PADH + tj
                    nc.tensor.matmul(
                        conv_ps[:, j * 2 * N:(j + 1) * 2 * N],
                        lhsT=xs_sb[:, base:base + W],
                        rhs=wp_sb[:, t * 2 * N:(t + 1) * 2 * N],
                        start=(t == 0),
                        stop=(t == N - 1),
                    )
            # ---- 2. positions / fractions / gather indices (f32) ----
            # offsets (incl. bias) read straight from conv PSUM
            offs3 = conv_ps[:].rearrange("p (j s) -> p j s", j=JROWS)
            cpx3 = cpx_sb[:].rearrange("p (j n) -> p j n", j=JROWS)
            cpy3 = cpy_sb[:].rearrange("p (j n) -> p j n", j=JROWS)

            px = smalls.tile([128, NSAMP], F32)
            px3 = px[:].rearrange("p (j n) -> p j n", j=JROWS)
            # px = offx + (4g+1) + cpx   (cpx carries h0 + pnx[n] + j from host)
            nc.vector.scalar_tensor_tensor(
                px3, offs3[:, :, 0:N], float(JROWS * g + 1), cpx3, OP.add, OP.add)
            py = smalls.tile([128, NSAMP], F32)
            py3 = py[:].rearrange("p (j n) -> p j n", j=JROWS)
            nc.vector.tensor_tensor(py3, offs3[:, :, N:2 * N], cpy3, OP.add)

            nc.vector.tensor_scalar(px[:], px[:], 129.0, 0.0, OP.min, OP.max)
            nc.vector.tensor_scalar(py[:], py[:], 129.0, 0.0, OP.min, OP.max)

            def floor_of(src):
                # HW f32->i32 copy rounds to nearest (CoreSim truncates), so
                # round then subtract 1 where the result overshot.
                fi = tmp.tile([128, NSAMP], I32, name="fi")
                nc.vector.tensor_copy(fi[:], src[:])
                ff = smalls.tile([128, NSAMP], F32, name="ff")
                nc.vector.tensor_copy(ff[:], fi[:])
                gt = tmp.tile([128, NSAMP], F32, name="gt")
                nc.vector.tensor_tensor(gt[:], ff[:], src[:], OP.is_gt)
                nc.vector.tensor_tensor(ff[:], ff[:], gt[:], OP.subtract)
                return ff

            flr = floor_of(px)
            flc = floor_of(py)

            # Broadcast-weight tile W[p, s, k, c] = w_k(p, s) for the three
            # difference-table lanes (k=0: b'*fc, k=1: c'*fr, k=2: d'*fr*fc),
            # built by writing the fractions into c=0 and log-doubling along c
            # (packed bf16 copies run at 4x on DVE).
            # 16-wide is enough: the multiply below runs in four c-quarters
            # all sharing the same weight columns.
            WTC = 16
            wt = sb.tile([128, NSAMP, 3, WTC], BF16, name="wt")
            nc.vector.tensor_tensor(
                wt[:, :, 0, 0], py[:], flc[:], OP.subtract)
            nc.vector.tensor_tensor(
                wt[:, :, 1, 0], px[:], flr[:], OP.subtract)
            nc.vector.tensor_tensor(
                wt[:, :, 2, 0], wt[:, :, 0, 0], wt[:, :, 1, 0], OP.mult)
            m = 1
            while m < WTC:
                nc.vector.tensor_copy(
                    wt[:, :, :, m:2 * m], wt[:, :, :, 0:m])
                m *= 2

            idx_f = smalls.tile([128, NSAMP], F32)
            nc.vector.scalar_tensor_tensor(
                idx_f[:], flr[:], float(PADH), flc[:], OP.mult, OP.add)

            # ---- 3. wrap indices for dma_gather: sample i=(t*128+16u+q) sits
            # at wrapped position [q, t*8+u]; build via two PE transposes and a
            # selector matmul that also replicates across the 8 Q7 cores.
            # merged PSUM tile (one 2KB bank): cols 0:128 hold idxT on
            # partitions 0:36; cols 128:416 hold the 16-wide re-transposes on
            # partitions 0:16 (compiler requires transpose out partition 0)
            tps = ps_idx.tile([NSAMP, 128 + 8 * NSAMP], F32)
            nc.tensor.transpose(tps[0:NSAMP, 0:128], idx_f[:], identf[:])
            idxT = smalls.tile([NSAMP, 128], F32)
            nc.scalar.copy(idxT[:], tps[0:NSAMP, 0:128])
            for u in range(8):
                nc.tensor.transpose(
                    tps[0:16, 128 + u * NSAMP:128 + (u + 1) * NSAMP],
                    idxT[:, 16 * u:16 * u + 16], identf[0:NSAMP, 0:NSAMP])
            w16 = smalls.tile([16, 8 * NSAMP], F32)
            # w16[q, t*8+u] = tps[q, 128 + u*36+t]
            nc.scalar.copy(
                w16[:].rearrange("p (t u) -> p u t", u=8),
                tps[0:16, 128:].rearrange("p (u t) -> p u t", u=8))
            rep_ps = ps_idx.tile([128, 8 * NSAMP], F32)
            nc.tensor.matmul(rep_ps[:], lhsT=sel_sb[:], rhs=w16[:], start=True, stop=True)
            idx16 = smalls.tile([128, 8 * NSAMP], mybir.dt.int16)
            nc.scalar.copy(idx16[:], rep_ps[:])

            # gather rows declared as int32 (same bytes): the table row is
            # 512B either way, this just matches the DMA element granularity
            gath = gpool.tile([128, NSAMP, 2 * C], I32)
            for half in range(2):
                s0, s1 = half * (NSAMP // 2), (half + 1) * (NSAMP // 2)
                nc.gpsimd.dma_gather(
                    gath[:, s0:s1, :], xt[:],
                    idx16[:, s0 * 8:s1 * 8],
                    num_idxs=128 * (s1 - s0), num_idxs_reg=128 * (s1 - s0),
                    elem_size=2 * C, elem_step=2 * C,
                    single_packet=False,
                    queue_num=half,
                )

            # ---- 4. bilinear combine: one big multiply + one add on DVE.
            # The remaining two summands (d'-lane product and the a corner)
            # fold into the PE transposes below via PSUM accumulation.
            # pp is lane-major so the per-lane sample-pair chunks fed to the
            # accumulating transposes are contiguous 128-column slices.
            gv = gath[:].bitcast(BF16).rearrange("p s (k c) -> p s k c", c=C)
            pp = sb.tile([128, 4, NSAMP, C], BF16, name="pp")

            if dbg and g == 0:
                nc.sync.dma_start(dbg["idx16"][:], idx16[:])

            # ---- 5. per j-row: combine slice, transposes, final conv ----
            out_sb = sb.tile([OUTC, JROWS * W], F32)
            for j in range(JROWS):
                ss = slice(j * N, (j + 1) * N)
                # last group: Pool is idle after its final gather, so it takes
                # rows 2-3 of the combine to halve the pipeline-drain tail
                last = g == NGROUP - 1
                veng = nc.gpsimd if j == 3 else nc.vector
                if j == 0:
                    nc.gpsimd.tensor_copy(pp[:, 0, ss, :], gv[:, ss, 0, :])
                else:
                    veng.tensor_copy(pp[:, 0, ss, :], gv[:, ss, 0, :])
                for q4 in range(C // 16):
                    cs = slice(16 * q4, 16 * q4 + 16)
                    veng.tensor_tensor(
                        pp[:, 1:4, ss, cs].rearrange("p k s c -> p s k c"),
                        gv[:, ss, 1:4, cs], wt[:, ss, :, :], OP.mult)

                # one f32 PSUM tile: cols 0:640 hold the transposed samples,
                # cols 640:768 the final-conv accumulator (2 banks)
                jps = ps_xot.tile([128, 6 * 128], F32)
                for q in range(5):
                    sl = slice(j * N + 2 * q, j * N + min(2 * q + 2, N))
                    dst = jps[:, q * 128:(q + 1) * 128] if q < 4 else \
                        jps[0:C, 4 * 128:5 * 128]
                    # xoT chunk = T(a) + sum_k T(w_k * lane_k): transposes as
                    # regular matmuls (lhsT=data, rhs=identity) because the
                    # HW transpose datapath does not accumulate in PSUM
                    for k in range(4):
                        nc.tensor.matmul(
                            dst, lhsT=pp[:, k, sl, :], rhs=ident[:],
                            start=(k == 0), stop=(k == 3))
                xot = sb.tile([128, 4 * 128], BF16, name="xot")
                xot8 = sb.tile([C, 128], BF16, name="xot8")
                if last and j in (1, 2):
                    nc.vector.tensor_copy(xot[:], jps[:, 0:4 * 128])
                    nc.vector.tensor_copy(xot8[:], jps[0:C, 4 * 128:5 * 128])
                else:
                    nc.scalar.copy(xot[:], jps[:, 0:4 * 128])
                    nc.scalar.copy(xot8[:], jps[0:C, 4 * 128:5 * 128])
                if dbg and g == 0 and j == 0:
                    nc.sync.dma_start(dbg["xot"][:], xot[:])

                o_ps = jps[0:OUTC, 5 * 128:6 * 128]
                for q in range(4):
                    nc.tensor.matmul(
                        o_ps,
                        lhsT=wc2_sb[:, q * OUTC:(q + 1) * OUTC],
                        rhs=xot[:, q * 128:(q + 1) * 128],
                        start=(q == 0),
                        stop=False,
                    )
                nc.tensor.matmul(
                    o_ps, lhsT=wc8_sb[:], rhs=xot8[:], start=False, stop=True)
                nc.scalar.copy(out_sb[:, j * W:(j + 1) * W], o_ps)
                nc.sync.dma_start(
                    out[:, (g * JROWS + j) * W:(g * JROWS + j + 1) * W],
                    out_sb[:, j * W:(j + 1) * W])
            if dbg and g == 0:
                nc.sync.dma_start(dbg["gath"][:].rearrange("p (s e) -> p s e", e=2*C).bitcast(mybir.dt.int32), gath[:])
                nc.sync.dma_start(dbg["wt"][:].rearrange("p (s k c) -> p s k c", k=3, c=16), wt[:])
                nc.sync.dma_start(dbg["pp"][:].rearrange("p (k s c) -> p k s c", s=NSAMP, c=C), pp[:])


def _host_prep(x, Wp, bp, Wc):
    x = np.asarray(x, np.float32)
    Wp = np.asarray(Wp, np.float32)
    bp = np.asarray(bp, np.float32)
    Wc = np.asarray(Wc, np.float32)
    bf16 = ml_dtypes.bfloat16

    # difference tables per batch
    tables = []
    for b in range(B):
        xp2 = np.pad(x[b], ((0, 0), (1, 2), (1, 2)))  # [C, 131, 131]
        a = xp2[:, :PADH, :PADH]
        r = xp2[:, :PADH, 1:PADH + 1]
        d = xp2[:, 1:PADH + 1, :PADH]
        dg = xp2[:, 1:PADH + 1, 1:PADH + 1]
        comp = np.stack([a, r - a, d - a, a + dg - r - d], axis=0)  # [4,C,130,130]
        t = comp.transpose(2, 3, 0, 1).reshape(TROWS, 4 * C)
        tables.append(np.ascontiguousarray(t.astype(bf16)).view(np.int32))

    # wp[c, t*18+m] = Wp[m, c, t//3, t%3]; row C carries the bias in tap 0
    wp_r = np.zeros((C + 1, N, 2 * N), np.float32)
    wp_r[:C] = Wp.reshape(2 * N, C, N).transpose(1, 2, 0)
    wp_r[C, 0, :] = bp
    wp_r = np.ascontiguousarray(wp_r.reshape(C + 1, N * 2 * N).astype(bf16))

    nidx = np.arange(N)
    pnx = (nidx // 3 - 1).astype(np.float32)
    pny = (nidx % 3 - 1).astype(np.float32)
    p = np.arange(128, dtype=np.float32)
    cpx0 = np.zeros((128, NSAMP), np.float32)
    cpy = np.zeros((128, NSAMP), np.float32)
    for j in range(JROWS):
        cpx0[:, j * N:(j + 1) * N] = pnx[None, :] + j
        cpy[:, j * N:(j + 1) * N] = pny[None, :] + (p[:, None] + 1.0)

    wcf = Wc.reshape(OUTC, C, N)
    wc2 = np.zeros((2 * C, 4 * OUTC), np.float32)
    for q in range(4):
        for s_ in range(2):
            wc2[s_ * C:(s_ + 1) * C, q * OUTC:(q + 1) * OUTC] = wcf[:, :, 2 * q + s_].T
    wc8 = np.ascontiguousarray(wcf[:, :, 8].T)

    sel_m = (np.arange(128)[None, :] % 16 == np.arange(16)[:, None]).astype(np.float32)

    xp1 = [np.pad(x[b], ((0, 0), (1, 1), (1, 1))) for b in range(B)]

    in_maps = []
    for k in range(8):
        bk, h0 = k // 4, (k % 4) * ROWS_PER_CORE
        xs_slice = xp1[bk][:, h0:h0 + XS_ROWS, :].reshape(C, XS_ROWS * PADH)
        xs2 = np.concatenate(
            [xs_slice, np.ones((1, XS_ROWS * PADH), np.float32)], axis=0)
        in_maps.append({
            "xt": tables[bk],
            "xs": np.ascontiguousarray(xs2.astype(bf16)),
            "wp": wp_r,
            "cpx": cpx0 + np.float32(h0),
            "cpy": cpy,
            "wc2": np.ascontiguousarray(wc2.astype(bf16)),
            "wc8": wc8.astype(bf16),
            "sel": sel_m,
        })
    return in_maps


def kernel(x, Wp, bp, Wc):
    if "nc" not in _CACHE:
        _CACHE["nc"] = _build_nc()
    nc = _CACHE["nc"]
    in_maps = _host_prep(x, Wp, bp, Wc)
    res = run_bass_kernel_spmd(nc, in_maps, list(range(8)))
    _CACHE["exec_time_ns"] = res.exec_time_ns
    _CACHE["results"] = res
    out = np.zeros((B, OUTC, H, W), np.float32)
    for k in range(8):
        bk, h0 = k // 4, (k % 4) * ROWS_PER_CORE
        out[bk, :, h0:h0 + ROWS_PER_CORE, :] = res.results[k]["out"].reshape(
            OUTC, ROWS_PER_CORE, W)
    return out

